# revision 37
# baseline (speedup 1.0000x reference)
"""TRN2 Bass kernel for nn_MultiBlockStructuredScoreNet.

Computes s(z) = -grad_z U(z) where
  U(z) = sum_k MLP_k(z_k) + sum_r z_8^T W_r z_{8-r}
for z of shape (8192, 9*256), data-parallel over 8 NeuronCores.

Per core (1024 samples):
 - Host pre-transposes z to neuron-major (zT) so the PE contracts over the
   neuron dim with no on-chip transposes; host packs/transposes/sign-flips
   the small parameters so PSUM accumulation directly yields the score.
 - MM_MODE picks the PE dtype: fp16 (default: full PE rate, ~2.5e-4),
   bf16 (~2e-3), or f32r (fp32 storage, ~1.3e-4 but slower weight loads).
 - Inputs ship as 4 packed DRAM tensors -> ~10 big DMAs (per-DMA overhead
   on the HWDGE queues is ~0.6us, so DMA count matters more than bytes).
 - A short burst of throwaway matmuls right after the params DMA warms the
   PE clock gate (HAM) before the real work lands.
 - MLP: u1 via col-tiled [K=128,M=32] matmuls packing 4 blocks per PSUM
   tile (f32r falls back to stacked zero-padded weights: its matmuls must
   write PSUM partition 0); u2/dh1 via block-diagonal [128,128] weights
   with -gW3 pre-folded into the dh1 weights; SiLU / SiLU' from the ACT
   table with fused bias, batched by function to avoid table reloads.
 - Cross couplings + MLP d_z accumulate b-major into PSUM per 128-sample
   chunk; adjacent blocks are paired into single N=512 matmuls (d_lag via
   adjacent wlag packing, d_z via block-diagonal W1^T pairs) to halve the
   weight-load count.  PSUM->SBUF copies split across DVE/ACT; one 1.2MB
   DMA per chunk stores the final rows.
"""

import numpy as np
import ml_dtypes

import concourse.bass as bass
import concourse.tile as tile
from concourse import bacc, mybir
from concourse.bass_utils import run_bass_kernel_spmd

AF = mybir.ActivationFunctionType
F32 = mybir.dt.float32

N_CORES = 8
BATCH = 8192
B_CORE = BATCH // N_CORES     # 1024
BT = 512                      # batch tile (PSUM free-dim max for f32)
NBT = B_CORE // BT            # 2 batch tiles per core
NCHUNK = 4                    # 128-sample chunks per batch tile
NB = 9                        # blocks
P_MAX = 8
NN = 256                      # neurons per block
D = NB * NN                   # 2304
H = 32

ZW = 2 * NB * BT              # zt tile cols per batch tile: 18 chunks x 512
OFF_W1T = 2 * NB * 128        # 2304: paired block-diag W1^T region
OFF_W2BD = OFF_W1T + 4 * 512 + 256   # 4608
OFF_W2TBD = OFF_W2BD + 3 * 128       # 4992
PW = OFF_W2TBD + 3 * 128             # 5376
OFF_WFUT = 2 * P_MAX * NN     # 4096
CW = 2 * OFF_WFUT             # 8192

MM_MODE = "fp16"              # "fp16" | "bf16" | "f32r"
WARMUP_MMS = 12

_DT = {
    "fp16": (mybir.dt.float16, np.float16),
    "bf16": (mybir.dt.bfloat16, ml_dtypes.bfloat16),
    "f32r": (mybir.dt.float32r, np.float32),
}


def _body(tc, out, zt, params, cparams, biases, ctx):
    nc = tc.nc
    sdt = _DT[MM_MODE][0]

    const = ctx.enter_context(tc.tile_pool(name="const", bufs=1))
    ztp = ctx.enter_context(tc.tile_pool(name="ztp", bufs=2))
    mlpp = ctx.enter_context(tc.tile_pool(name="mlpp", bufs=3, space="PSUM"))
    actp = ctx.enter_context(tc.tile_pool(name="actp", bufs=8))
    du1p = ctx.enter_context(tc.tile_pool(name="du1p", bufs=6))
    outp = ctx.enter_context(tc.tile_pool(name="outp", bufs=5, space="PSUM"))
    outs = ctx.enter_context(tc.tile_pool(name="outs", bufs=4))

    # ---- inputs -> SBUF (few big DMAs; ordered so compute starts early)
    pa_sb = const.tile([128, PW], sdt, name="pa")
    nc.sync.dma_start(pa_sb[:], params[:])
    bias_sb = const.tile([128, 6], F32, name="biassb")
    nc.sync.dma_start(bias_sb[:], biases[:])
    zt_sb = [ztp.tile([128, ZW], sdt, tag="zt", name="ztsb") for _ in range(NBT)]
    # arrival order tracks PE consumption order: z_future chunks (16-17)
    # first (every d_lag matmul reads them), then the MLP chunks, then the
    # coupling matrices, then batch tile 1.
    cp_sb = const.tile([128, CW], sdt, name="cp")
    nc.sync.dma_start(zt_sb[0][:, 16 * BT:ZW], zt[0, :, 16 * BT:ZW])
    for a, b in [(0, 8 * BT), (8 * BT, 16 * BT)]:
        nc.sync.dma_start(zt_sb[0][:, a:b], zt[0, :, a:b])
    nc.sync.dma_start(cp_sb[:, :OFF_WFUT], cparams[:, :OFF_WFUT])
    nc.sync.dma_start(cp_sb[:, OFF_WFUT:], cparams[:, OFF_WFUT:])
    for a, b in [(0, 8 * BT), (8 * BT, 16 * BT), (16 * BT, ZW)]:
        nc.sync.dma_start(zt_sb[1][:, a:b], zt[1, :, a:b])

    def ztsl(t, c, c0, w):    # [128, w] slice at offset c0 of zT chunk c
        return zt_sb[t][:, BT * c + c0:BT * c + c0 + w]

    # ---- HAM warm-up: throwaway matmuls on the params tile, landing just
    # before the first real matmuls so the PE clock gate stays open into
    # the dense phase (an earlier warm-up decays during the DMA lead-in).
    wu = mlpp.tile([128, BT], F32, tag="mlpp", name="wut")
    for i in range(WARMUP_MMS):
        nc.tensor.matmul(wu[:], pa_sb[:, 0:128], pa_sb[:, 0:BT],
                         start=True, stop=True)

    u1_state = {}
    act_state = {}
    du1_tiles = {}

    def u1_group(t, g):
        nblk = 4 if g < 2 else 1
        P = 32 * nblk
        u1 = mlpp.tile([128, BT], F32, tag="mlpp", name="u1t")
        for j in range(nblk):
            k = 4 * g + j
            for hf in range(2):
                c = 2 * k + hf
                if MM_MODE == "f32r":
                    nc.tensor.matmul(
                        u1[:P, :], pa_sb[:, 128 * c:128 * c + P],
                        ztsl(t, c, 0, BT),
                        start=(2 * j + hf == 0), stop=(2 * j + hf == 2 * nblk - 1))
                else:
                    nc.tensor.matmul(
                        u1[32 * j:32 * j + 32, :],
                        pa_sb[:, 128 * c + 32 * j:128 * c + 32 * j + 32],
                        ztsl(t, c, 0, BT), start=(hf == 0), stop=(hf == 1),
                        tile_position=(0, 32 * j))
        return u1, P

    def u1_phase(t, order=(0, 1, 2)):
        u1s = [None] * 3
        for g in order:
            u1s[g] = u1_group(t, g)
        u1_state[t] = u1s

    def mlp_act_phase(t):
        # SiLU + SiLU' on ACT only -- emitted early so the ACT engine can
        # run these under cross-phase PE work of the previous batch tile.
        u1s = u1_state[t]
        h1s, sp1s = [], []
        for g in range(3):
            u1, P = u1s[g]
            h1 = actp.tile([128, BT], sdt, tag="act", name="h1t")
            nc.scalar.activation(h1[:P], u1[:P], AF.Silu, bias=bias_sb[:P, g:g + 1])
            h1s.append(h1)
        for g in range(3):
            u1, P = u1s[g]
            sp1 = actp.tile([128, BT], F32, tag="act", name="sp1t")
            nc.scalar.activation(sp1[:P], u1[:P], AF.Derivative_silu,
                                 bias=bias_sb[:P, g:g + 1])
            sp1s.append(sp1)
        act_state[t] = (h1s, sp1s)

    bwd_state = {}

    def mlp_u2_phase(t):
        # u2 matmuls + sp2: first half of the backward chain
        h1s, sp1s = act_state[t]
        u1s = u1_state[t]
        u2s, sp2s = [], []
        for g in range(3):
            P = u1s[g][1]
            u2 = mlpp.tile([128, BT], F32, tag="mlpp", name="u2t")
            nc.tensor.matmul(u2[:P], pa_sb[:P, OFF_W2BD + 128 * g:OFF_W2BD + 128 * g + P],
                             h1s[g][:P], start=True, stop=True)
            u2s.append(u2)
        for g in range(3):
            P = u1s[g][1]
            sp2 = actp.tile([128, BT], sdt, tag="act", name="sp2t")
            nc.scalar.activation(sp2[:P], u2s[g][:P], AF.Derivative_silu,
                                 bias=bias_sb[:P, 3 + g:4 + g])
            sp2s.append(sp2)
        bwd_state[t] = sp2s

    def mlp_bwd_phase(t):
        # dh1 matmuls + du1: second half of the backward chain
        h1s, sp1s = act_state[t]
        u1s = u1_state[t]
        sp2s = bwd_state[t]
        du1_sb = []
        for g in range(3):
            P = u1s[g][1]
            dh1 = mlpp.tile([128, BT], F32, tag="mlpp", name="dh1t")
            # -gW3 is folded into these weights: dh1 here is -d(e)/d(h1)
            nc.tensor.matmul(dh1[:P],
                             pa_sb[:P, OFF_W2TBD + 128 * g:OFF_W2TBD + 128 * g + P],
                             sp2s[g][:P], start=True, stop=True)
            du1 = du1p.tile([128, BT], sdt, tag="du1", name="du1t")
            nc.vector.tensor_mul(du1[:P], dh1[:P], sp1s[g][:P])
            du1_sb.append(du1)
        du1_tiles[t] = du1_sb

    def cross_chunk(t, c, dve_only=False, last=False, pipelined=False):
        du1_sb = du1_tiles[t]
        bs = slice(128 * c, 128 * c + 128)
        ot = outs.tile([128, D], F32, tag="outs", name="outst")
        op = [outp.tile([128, 512], F32, tag="outp", name="outpt")
              for _ in range(4)]
        o8 = outp.tile([128, 512], F32, tag="outp", name="outpt")

        def dlag(p):
            for ih in range(2):
                nc.tensor.matmul(
                    op[p][:], ztsl(t, 2 * P_MAX + ih, 128 * c, 128),
                    cp_sb[:, 2048 * ih + 512 * p:2048 * ih + 512 * p + 512],
                    start=(ih == 0), stop=False)

        def dz(p):
            base = 64 * (p % 2)
            nc.tensor.matmul(
                op[p][:], du1_sb[p // 2][base:base + 64, bs],
                pa_sb[base:base + 64, OFF_W1T + 512 * p:OFF_W1T + 512 * p + 512],
                start=False, stop=True, tile_position=(base, 0))

        def copy(p, eng):
            if eng == 'v':
                nc.vector.tensor_copy(ot[:, 512 * p:512 * (p + 1)], op[p][:])
            else:
                nc.scalar.activation(ot[:, 512 * p:512 * (p + 1)], op[p][:], AF.Copy)

        def dfut():
            for r in range(1, P_MAX + 1):
                for jh in range(2):
                    i = 2 * (r - 1) + jh
                    nc.tensor.matmul(
                        o8[:, :NN], ztsl(t, 2 * (P_MAX - r) + jh, 128 * c, 128),
                        cp_sb[:, OFF_WFUT + 256 * i:OFF_WFUT + 256 * i + 256],
                        start=(i == 0), stop=False)
            nc.tensor.matmul(o8[:, :NN], du1_sb[2][0:32, bs],
                             pa_sb[0:32, OFF_W1T + 2048:OFF_W1T + 2048 + 256],
                             start=False, stop=True, tile_position=(0, 0))

        r0 = t * BT + c * 128
        if pipelined:
            for p in range(4):
                dlag(p)
                dz(p)
                copy(p, 'v' if p % 2 == 0 else 's')
            dfut()
            nc.vector.tensor_copy(ot[:, 2048:2304], o8[:, :NN])
            if last:
                nc.sync.dma_start(out[r0:r0 + 128, 0:1024], ot[:, 0:1024])
                nc.sync.dma_start(out[r0:r0 + 128, 1024:2304], ot[:, 1024:2304])
                return
        else:
            for p in range(4):
                dlag(p)
            dfut()
            for p in range(4):
                dz(p)
            for p in range(3):
                copy(p, 'v')
            if dve_only:
                copy(3, 'v')
                nc.vector.tensor_copy(ot[:, 2048:2304], o8[:, :NN])
            else:
                copy(3, 's')
                nc.scalar.activation(ot[:, 2048:2304], o8[:, :NN], AF.Copy)
        nc.sync.dma_start(out[r0:r0 + 128, :], ot[:])

    u1_phase(0, order=(2, 0, 1))     # g2 first: its zT chunks arrive first
    mlp_act_phase(0)
    mlp_u2_phase(0)
    mlp_bwd_phase(0)
    cross_chunk(0, 0)
    cross_chunk(0, 1)
    u1_phase(1)
    mlp_act_phase(1)
    cross_chunk(0, 2, dve_only=True)
    mlp_u2_phase(1)
    cross_chunk(0, 3)
    mlp_bwd_phase(1)
    cross_chunk(1, 0, pipelined=True)
    cross_chunk(1, 1, pipelined=True)
    cross_chunk(1, 2, pipelined=True)
    cross_chunk(1, 3, last=True, pipelined=True)


# ------------------------------------------------------------- build + launch

_CACHED = {}


def _build():
    if MM_MODE in _CACHED:
        return _CACHED[MM_MODE]
    sdt = _DT[MM_MODE][0]
    nc = bacc.Bacc("TRN2", target_bir_lowering=False, debug=False,
                   num_devices=N_CORES)
    zt = nc.dram_tensor("zt", [NBT, 128, ZW], sdt, kind="ExternalInput").ap()
    params = nc.dram_tensor("params", [128, PW], sdt, kind="ExternalInput").ap()
    cparams = nc.dram_tensor("cparams", [128, CW], sdt, kind="ExternalInput").ap()
    biases = nc.dram_tensor("biases", [128, 6], F32, kind="ExternalInput").ap()
    out = nc.dram_tensor("out", [B_CORE, D], F32, kind="ExternalOutput").ap()

    from contextlib import ExitStack
    with tile.TileContext(nc) as tc:
        with ExitStack() as ctx:
            _body(tc, out, zt, params, cparams, biases, ctx)
    nc.compile()
    _CACHED[MM_MODE] = nc
    return nc


def _prep_params(gW1, gb1, gW2, gb2, gW3, gb3, W):
    mdt = _DT[MM_MODE][1]
    params = np.zeros((128, PW), np.float32)
    biases = np.zeros((128, 6), np.float32)
    for k in range(NB):
        g, j = k // 4, k % 4
        rs = slice(32 * j, 32 * j + 32)
        for hf in range(2):
            # u1 lhsT piece (2k+hf): cols 32j..32j+32 hold W1_k[half]
            params[:, 128 * (2 * k + hf) + 32 * j:128 * (2 * k + hf) + 32 * j + 32] = \
                gW1[k, 128 * hf:128 * (hf + 1), :]
        params[rs, OFF_W2BD + 128 * g + 32 * j:OFF_W2BD + 128 * g + 32 * j + 32] = gW2[k]
        # dh1 weights with -gW3 folded in:
        # lhsT[32j+g', 32j+h] = -gW3[k][g'] * gW2[k][h, g']
        params[rs, OFF_W2TBD + 128 * g + 32 * j:OFF_W2TBD + 128 * g + 32 * j + 32] = \
            -gW3[k][:, None] * gW2[k].T
        biases[rs, g] = gb1[k]
        biases[rs, 3 + g] = gb2[k]
    # paired block-diagonal W1^T for the d_z matmuls (not negated: du1
    # already carries the sign flip from the folded -gW3)
    for m in range(4):
        base = 64 * (m % 2)
        for s in range(2):
            k = 2 * m + s
            params[base + 32 * s:base + 32 * s + 32,
                   OFF_W1T + 512 * m + 256 * s:OFF_W1T + 512 * m + 256 * (s + 1)] = \
                gW1[k].T
    params[0:32, OFF_W1T + 2048:OFF_W1T + 2048 + 256] = gW1[8].T

    cparams = np.zeros((128, CW), np.float32)
    for ih in range(2):
        for k in range(P_MAX):            # block k pairs with lag r = 8-k
            cparams[:, 2048 * ih + 256 * k:2048 * ih + 256 * (k + 1)] = \
                -W[7 - k][128 * ih:128 * (ih + 1), :]
    for r in range(1, P_MAX + 1):
        for jh in range(2):
            i = 2 * (r - 1) + jh
            cparams[:, OFF_WFUT + 256 * i:OFF_WFUT + 256 * (i + 1)] = \
                -W[r - 1].T[128 * jh:128 * (jh + 1), :]
    return {"params": params.astype(mdt), "cparams": cparams.astype(mdt),
            "biases": biases}


def run(inputs, trace=False):
    nc = _build()
    mdt = _DT[MM_MODE][1]
    params = _prep_params(
        np.asarray(inputs["gW1"]), np.asarray(inputs["gb1"]),
        np.asarray(inputs["gW2"]), np.asarray(inputs["gb2"]),
        np.asarray(inputs["gW3"]), np.asarray(inputs["gb3"]),
        np.asarray(inputs["W"]))
    z = np.asarray(inputs["z"])
    in_maps = []
    for ci in range(N_CORES):
        zc = z[ci * B_CORE:(ci + 1) * B_CORE]
        # zt[t, p, 512*c + b] = zc[512t + b, 128c + p]
        ztc = np.ascontiguousarray(
            zc.reshape(NBT, BT, 2 * NB, 128).transpose(0, 3, 2, 1)
        ).reshape(NBT, 128, ZW).astype(mdt)
        in_maps.append({"zt": ztc, **params})
    res = run_bass_kernel_spmd(nc, in_maps, core_ids=list(range(N_CORES)),
                               trace=trace)
    out = np.concatenate([r["out"] for r in res.results], axis=0)
    return out, res


def kernel(**inputs) -> np.ndarray:
    out, _ = run(inputs, trace=False)
    return out
